# revision 10
# baseline (speedup 1.0000x reference)
"""Distributed Trainium2 kernel for the Ada_GCNResnet block.

Sharding: sequence-parallel over the N = B*H*W = 3136 graph nodes, 392 nodes
(= 2 images) per core.  Each core computes its rows of feat / adj / gc, with
three AllGathers (feat, support1, support2) providing the full tensors needed
for the adjacency contractions.  GCN weights are replicated.

Precision: bf16 matmuls with fp32 PSUM accumulation everywhere except the
row-sum path of the adjacency normalization, which is ill-conditioned
(min |row_sum| ~ 0.5 vs typical ~900) and computed in exact fp32 via the
identity  row_sum[r] = X[:,r]. t + b.s  with  t = WdT @ s,
s = WdT^T @ u + N*b,  u = sum_n X[:,n].
"""

import sys

sys.path.insert(0, "/opt/trn_rl_repo")

import numpy as np
import ml_dtypes

from concourse import bacc, tile, mybir
from concourse.bass_utils import run_bass_kernel_spmd

NCORES = 8
B, C, D, HWP = 16, 2048, 1024, 196
N = 3136          # B * 14 * 14 nodes
R = N // NCORES   # 392 local nodes = 2 images
NCLS = 80
KC = C // 128     # 16
KD = D // 128     # 8
MN = (N + 127) // 128  # 25 n-tiles (last has 64 rows)

F32 = mybir.dt.float32
BF16 = mybir.dt.bfloat16
RG = [list(range(NCORES))]

_BUILT = None


def _build():
    nc = bacc.Bacc("TRN2", target_bir_lowering=False, debug=False,
                   num_devices=NCORES)

    dp = nc.declare_dram_parameter
    xbf_d = dp("xbf", [C, R], BF16, isOutput=False)
    x32_d = dp("x32", [C, R], F32, isOutput=False)
    wdT_d = dp("wdT", [C, D], BF16, isOutput=False)
    w1_d = dp("w1", [D, D], BF16, isOutput=False)
    w2_d = dp("w2", [D, D], BF16, isOutput=False)
    wuT_d = dp("wuT", [D, C], BF16, isOutput=False)
    wf_d = dp("wfT", [C, NCLS], F32, isOutput=False)
    bd_d = dp("bd", [D, 1], F32, isOutput=False)
    b1_d = dp("b1", [D, 1], F32, isOutput=False)
    b2_d = dp("b2", [D, 1], F32, isOutput=False)
    bnA_d = dp("bnA", [C, 1], F32, isOutput=False)
    bnB_d = dp("bnB", [C, 1], F32, isOutput=False)
    bfc_d = dp("bfc", [NCLS, 1], F32, isOutput=False)
    t_d = dp("t32", [C, 1], F32, isOutput=False)
    bs_d = dp("bs", [1, 1], F32, isOutput=False)
    out_d = dp("out", [NCLS, 2], F32, isOutput=True)

    with tile.TileContext(nc) as tc:
        with (
            tc.tile_pool(name="wpool", bufs=1) as wp,
            tc.tile_pool(name="main", bufs=1) as mp,
            tc.tile_pool(name="dram", bufs=1, space="DRAM") as dr,
        ):
            # ---- long-lived SBUF tensors
            w1_sb = wp.tile([128, KD, D], BF16)
            w2_sb = wp.tile([128, KD, D], BF16)
            wuT_sb = wp.tile([128, KD, C], BF16)
            wf_sb = wp.tile([128, KC, NCLS], F32)
            bd_sb = wp.tile([128, KD], F32)
            b1_sb = wp.tile([128, KD], F32)
            b2_sb = wp.tile([128, KD], F32)
            bnA_sb = wp.tile([128, KC], F32)
            bnB_sb = wp.tile([128, KC], F32)
            bfc_sb = wp.tile([NCLS, 1], F32)
            t_sb = wp.tile([128, KC], F32)
            bs_sb = wp.tile([1, 1], F32)
            ones_sb = wp.tile([1, 128], F32)

            feat_bf = mp.tile([128, KD, R], BF16)
            adjT_sc = mp.tile([128, MN, R], BF16)
            gc1T = mp.tile([128, KD, R], BF16)
            gc2T = mp.tile([128, KD, R], BF16)
            rinv_bc = mp.tile([128, R], F32)
            rs_row = mp.tile([1, R], F32)
            rinv_row = mp.tile([1, R], F32)
            pooled = mp.tile([128, KC, 2], F32)
            s_sb = mp.tile([128, 4, D], BF16)  # support1 / support2 (reused)
            out_sb = mp.tile([NCLS, 2], F32)

            # ---- DRAM bounce buffers for collectives
            feat_bnc = dr.tile([D, R], BF16)
            feat_g = dr.tile([NCORES * D, R], BF16, addr_space="Shared")
            s1_bnc = dr.tile([R, D], BF16)
            s1_g = dr.tile([N, D], BF16, addr_space="Shared")
            s2_bnc = dr.tile([R, D], BF16)
            s2_g = dr.tile([N, D], BF16, addr_space="Shared")

            # round-robin DMA issue across engine queues
            _eng = [nc.sync, nc.scalar]
            _ei = [0]

            def dma(*a, **k):
                e = _eng[_ei[0] % len(_eng)]
                _ei[0] += 1
                return e.dma_start(*a, **k)

            # ---- phase 0: first-wave input DMAs (chunked so P1 starts early)
            with tc.tile_pool(name="downp", bufs=1) as dnp:
                xbf_sb = dnp.tile([128, KC, R], BF16)
                x32_sb = dnp.tile([128, KC, R], F32)
                wdT_sb = dnp.tile([128, KC, D], BF16)

                dma(bd_sb[:], bd_d.ap().rearrange("(k p) one -> p (k one)", p=128))
                xbf_r = xbf_d.ap().rearrange("(k p) r -> p k r", p=128)
                wdT_r = wdT_d.ap().rearrange("(k p) d -> p k d", p=128)
                for k in range(KC):
                    dma(wdT_sb[:, k, :], wdT_r[:, k, :])
                    dma(xbf_sb[:, k, :], xbf_r[:, k, :])
                w1_r = w1_d.ap().rearrange("(k p) d -> p k d", p=128)
                for k in range(KD):
                    dma(w1_sb[:, k, :], w1_r[:, k, :])
                dma(b1_sb[:], b1_d.ap().rearrange("(k p) one -> p (k one)", p=128))
                nc.vector.memset(ones_sb[:], 1.0)

                # ---- phase 1: conv1x1-down  feat_T[d, r] (bf16)
                with tc.tile_pool(name="ps1", bufs=1, space="PSUM") as ps1:
                    for m in range(KD):
                        pd = ps1.tile([128, R], F32, tag="down", bufs=3)
                        for k in range(KC):
                            nc.tensor.matmul(pd[:], wdT_sb[:, k, 128 * m:128 * (m + 1)],
                                             xbf_sb[:, k, :],
                                             start=(k == 0), stop=(k == KC - 1))
                        with tc.high_priority():
                            nc.vector.tensor_scalar_add(feat_bf[:, m, :], pd[:],
                                                        bd_sb[:, m:m + 1])
                            nc.gpsimd.dma_start(feat_bnc[128 * m:128 * (m + 1), :],
                                                feat_bf[:, m, :])

                    # ---- AllGather 1: feat
                    with tc.high_priority():
                        nc.gpsimd.collective_compute(
                            "AllGather", mybir.AluOpType.bypass, replica_groups=RG,
                            ins=[feat_bnc[:].opt()], outs=[feat_g[:].opt()])

                    # ---- phase 3: support1 = feat @ w1 (local rows), overlaps AG1
                    for t in range(4):
                        r0 = 128 * t
                        wt = min(128, R - r0)
                        for h in range(2):
                            p = ps1.tile([128, 512], F32, tag="s1", bufs=4)
                            for k in range(KD):
                                nc.tensor.matmul(
                                    p[:wt], feat_bf[:, k, r0:r0 + wt],
                                    w1_sb[:, k, 512 * h:512 * (h + 1)],
                                    start=(k == 0), stop=(k == KD - 1))
                            with tc.high_priority(offset=5000):
                                nc.vector.tensor_copy(
                                    s_sb[:wt, t, 512 * h:512 * (h + 1)], p[:wt])
                        with tc.high_priority(offset=5000):
                            nc.gpsimd.dma_start(s1_bnc[r0:r0 + wt, :],
                                                s_sb[:wt, t, :])

                    with tc.high_priority(offset=5000):
                        nc.gpsimd.collective_compute(
                            "AllGather", mybir.AluOpType.bypass, replica_groups=RG,
                            ins=[s1_bnc[:].opt()], outs=[s1_g[:].opt()])

                    # exact fp32 row sums of the (unnormalized) adjacency
                    x32_r = x32_d.ap().rearrange("(k p) r -> p k r", p=128)
                    for k in range(KC):
                        dma(x32_sb[:, k, :], x32_r[:, k, :])
                    dma(t_sb[:], t_d.ap().rearrange("(k p) one -> p (k one)", p=128))
                    dma(bs_sb[:], bs_d.ap())
                    prs = ps1.tile([1, R], F32, tag="rsbc")
                    for k in range(KC):
                        nc.tensor.matmul(prs[:], t_sb[:, k:k + 1], x32_sb[:, k, :],
                                         start=(k == 0), stop=(k == KC - 1))
                    nc.vector.tensor_scalar_add(rs_row[:], prs[:], bs_sb[:1, :])
                    nc.vector.reciprocal(rinv_row[:], rs_row[:])
                    pbc = ps1.tile([128, R], F32, tag="rsbc")
                    nc.tensor.matmul(pbc[:], ones_sb[:], rinv_row[:],
                                     start=True, stop=True)
                    nc.vector.tensor_copy(rinv_bc[:], pbc[:])

                # prefetch late-phase weights into the AG1 window
                w2_r = w2_d.ap().rearrange("(k p) d -> p k d", p=128)
                wuT_r = wuT_d.ap().rearrange("(k p) c -> p k c", p=128)
                wf_r = wf_d.ap().rearrange("(k p) o -> p k o", p=128)
                for k in range(KD):
                    dma(w2_sb[:, k, :], w2_r[:, k, :])
                dma(b2_sb[:], b2_d.ap().rearrange("(k p) one -> p (k one)", p=128))
                for k in range(KD):
                    dma(wuT_sb[:, k, :], wuT_r[:, k, :])
                for k in range(KC):
                    dma(wf_sb[:, k, :], wf_r[:, k, :])
                dma(bnA_sb[:], bnA_d.ap().rearrange("(k p) one -> p (k one)", p=128))
                dma(bnB_sb[:], bnB_d.ap().rearrange("(k p) one -> p (k one)", p=128))
                dma(bfc_sb[:], bfc_d.ap())

                # ---- phase 4: adjT (row-scaled), needs gathered feat
                with (
                    tc.tile_pool(name="ps4", bufs=1, space="PSUM") as ps4,
                    tc.tile_pool(name="adjlhs", bufs=4) as alp,
                ):
                    feat_g2 = feat_g[:].rearrange("(j k p) r -> j p k r", j=NCORES, p=128)
                    for m in range(MN):
                        n0 = 128 * m
                        w = min(128, N - n0)
                        pa = ps4.tile([128, R], F32, tag="adj", bufs=3)
                        # one 3-D DMA per block segment covers all 8 k-tiles
                        lt = alp.tile([128, KD, 128], BF16, tag="lt")
                        j0 = n0 // R
                        j1 = (n0 + w - 1) // R
                        for j in range(j0, j1 + 1):
                            a = max(n0, R * j)
                            b = min(n0 + w, R * (j + 1))
                            dma(lt[:, :, a - n0:b - n0],
                                feat_g2[j, :, :, a - R * j:b - R * j])
                        for k in range(KD):
                            nc.tensor.matmul(pa[:w], lt[:, k, :w], feat_bf[:, k, :],
                                             start=(k == 0), stop=(k == KD - 1))
                        nc.vector.tensor_tensor(adjT_sc[:w, m, :], pa[:w],
                                                rinv_bc[:w, :],
                                                op=mybir.AluOpType.mult)

            # ---- phase 5: gc1_T = relu((adj_sc @ support1)^T + b1)
            with (
                tc.tile_pool(name="ps5", bufs=1, space="PSUM") as ps5,
                tc.tile_pool(name="gclhs1", bufs=4) as glp1,
            ):
                pg = [ps5.tile([128, R], F32, tag=f"gc{m}", name=f"pgc{m}")
                      for m in range(KD)]
                for k in range(MN):
                    wk = min(128, N - 128 * k)
                    kt = glp1.tile([128, D], BF16, tag="kt")
                    dma(kt[:wk], s1_g[128 * k:128 * k + wk, :])
                    for m in range(KD):
                        nc.tensor.matmul(pg[m][:], kt[:wk, 128 * m:128 * (m + 1)],
                                         adjT_sc[:wk, k, :],
                                         start=(k == 0), stop=(k == MN - 1))
                for m in range(KD):
                    nc.scalar.activation(gc1T[:, m, :], pg[m][:],
                                         mybir.ActivationFunctionType.Relu,
                                         bias=b1_sb[:, m:m + 1], scale=1.0)

            # ---- phase 6: support2 = gc1 @ w2 (local rows)
            with tc.tile_pool(name="ps6", bufs=1, space="PSUM") as ps6:
                for t in range(4):
                    r0 = 128 * t
                    wt = min(128, R - r0)
                    for h in range(2):
                        p = ps6.tile([128, 512], F32, tag="s2", bufs=4)
                        for k in range(KD):
                            nc.tensor.matmul(
                                p[:wt], gc1T[:, k, r0:r0 + wt],
                                w2_sb[:, k, 512 * h:512 * (h + 1)],
                                start=(k == 0), stop=(k == KD - 1))
                        nc.vector.tensor_copy(
                            s_sb[:wt, t, 512 * h:512 * (h + 1)], p[:wt])
                    nc.gpsimd.dma_start(s2_bnc[r0:r0 + wt, :], s_sb[:wt, t, :])

            nc.gpsimd.collective_compute(
                "AllGather", mybir.AluOpType.bypass, replica_groups=RG,
                ins=[s2_bnc[:].opt()], outs=[s2_g[:].opt()])

            # ---- phase 7: gc2_T = relu((adj_sc @ support2)^T + b2)
            with (
                tc.tile_pool(name="ps7", bufs=1, space="PSUM") as ps7,
                tc.tile_pool(name="gclhs2", bufs=4) as glp2,
            ):
                pg = [ps7.tile([128, R], F32, tag=f"gd{m}", name=f"pgd{m}")
                      for m in range(KD)]
                for k in range(MN):
                    wk = min(128, N - 128 * k)
                    kt = glp2.tile([128, D], BF16, tag="kt")
                    dma(kt[:wk], s2_g[128 * k:128 * k + wk, :])
                    for m in range(KD):
                        nc.tensor.matmul(pg[m][:], kt[:wk, 128 * m:128 * (m + 1)],
                                         adjT_sc[:wk, k, :],
                                         start=(k == 0), stop=(k == MN - 1))
                for m in range(KD):
                    nc.scalar.activation(gc2T[:, m, :], pg[m][:],
                                         mybir.ActivationFunctionType.Relu,
                                         bias=b2_sb[:, m:m + 1], scale=1.0)

            # ---- phase 8: conv1x1-up + BN + residual + maxpool
            with (
                tc.tile_pool(name="ps8", bufs=1, space="PSUM") as ps8,
                tc.tile_pool(name="upp", bufs=1) as up,
            ):
                for m in range(KC):
                    pu = ps8.tile([128, R], F32, tag="up", bufs=3)
                    for k in range(KD):
                        nc.tensor.matmul(pu[:], wuT_sb[:, k, 128 * m:128 * (m + 1)],
                                         gc2T[:, k, :],
                                         start=(k == 0), stop=(k == KD - 1))
                    xb = up.tile([128, R], F32, tag="xbn", bufs=3)
                    nc.scalar.activation(xb[:], pu[:],
                                         mybir.ActivationFunctionType.Identity,
                                         bias=bnB_sb[:, m:m + 1],
                                         scale=bnA_sb[:, m:m + 1])
                    x32u = up.tile([128, R], F32, tag="x32u", bufs=3)
                    dma(x32u[:], x32_d[128 * m:128 * (m + 1), :])
                    xr = up.tile([128, R], F32, tag="xres", bufs=3)
                    nc.vector.tensor_tensor(xr[:], xb[:], x32u[:],
                                            op=mybir.AluOpType.add)
                    nc.vector.tensor_reduce(
                        pooled[:, m, :], xr[:].rearrange("p (i q) -> p i q", i=2),
                        axis=mybir.AxisListType.X, op=mybir.AluOpType.max)

                # ---- fc
                pfc = ps8.tile([NCLS, 2], F32, tag="fc")
                for k in range(KC):
                    nc.tensor.matmul(pfc[:], wf_sb[:, k, :], pooled[:, k, :],
                                     start=(k == 0), stop=(k == KC - 1))
                nc.scalar.activation(out_sb[:], pfc[:],
                                     mybir.ActivationFunctionType.Identity,
                                     bias=bfc_sb[:], scale=1.0)
                dma(out_d[:], out_sb[:])

    nc.compile()
    return nc


def _prep(inputs):
    bf = ml_dtypes.bfloat16
    f = np.ascontiguousarray(inputs["feature"], dtype=np.float32)
    X = np.ascontiguousarray(f.transpose(1, 0, 2, 3).reshape(C, N))

    wdT = np.ascontiguousarray(inputs["w_down"].T, dtype=np.float32)
    b_down = inputs["b_down"].astype(np.float64)

    # exact row-sum folding: row_sum[r] = X[:,r].t + b.s
    u = X.sum(1, dtype=np.float64)
    s = wdT.astype(np.float64).T @ u + N * b_down
    t = wdT.astype(np.float64) @ s
    bs = float(b_down @ s) + 1e-10

    A = (inputs["bn_gamma"] / np.sqrt(inputs["bn_var"] + 1e-5)).astype(np.float32)
    Bb = (inputs["bn_beta"] + (inputs["b_up"] - inputs["bn_mean"]) * A).astype(np.float32)

    com = {
        "wdT": wdT.astype(bf),
        "w1": np.ascontiguousarray(inputs["w1"], dtype=np.float32).astype(bf),
        "w2": np.ascontiguousarray(inputs["w2"], dtype=np.float32).astype(bf),
        "wuT": np.ascontiguousarray(inputs["w_up"].T, dtype=np.float32).astype(bf),
        "wfT": np.ascontiguousarray(inputs["w_fc"].T, dtype=np.float32),
        "bd": inputs["b_down"].astype(np.float32).reshape(D, 1),
        "b1": inputs["b1"].astype(np.float32).reshape(D, 1),
        "b2": inputs["b2"].astype(np.float32).reshape(D, 1),
        "bnA": A.reshape(C, 1),
        "bnB": Bb.reshape(C, 1),
        "bfc": inputs["b_fc"].astype(np.float32).reshape(NCLS, 1),
        "t32": t.astype(np.float32).reshape(C, 1),
        "bs": np.full((1, 1), bs, dtype=np.float32),
    }
    in_maps = []
    for c in range(NCORES):
        xl = np.ascontiguousarray(X[:, R * c:R * (c + 1)])
        m = dict(com)
        m["x32"] = xl
        m["xbf"] = xl.astype(bf)
        in_maps.append(m)
    return in_maps


def kernel(**inputs):
    global _BUILT
    if _BUILT is None:
        _BUILT = _build()
    in_maps = _prep(inputs)
    res = run_bass_kernel_spmd(_BUILT, in_maps, core_ids=list(range(NCORES)))
    out = np.empty((B, NCLS), dtype=np.float32)
    for c in range(NCORES):
        o = res.results[c]["out"]  # (NCLS, 2)
        out[2 * c] = o[:, 0]
        out[2 * c + 1] = o[:, 1]
    return out


# revision 11
# speedup vs baseline: 1.0153x; 1.0153x over previous
"""Distributed Trainium2 kernel for the Ada_GCNResnet block.

Sharding: sequence-parallel over the N = B*H*W = 3136 graph nodes, 392 nodes
(= 2 images) per core.  Each core computes its rows of feat / adj / gc, with
three AllGathers (feat, support1, support2) providing the full tensors needed
for the adjacency contractions.  GCN weights are replicated.

Precision: bf16 matmuls with fp32 PSUM accumulation everywhere except the
row-sum path of the adjacency normalization, which is ill-conditioned
(min |row_sum| ~ 0.5 vs typical ~900) and computed in exact fp32 via the
identity  row_sum[r] = X[:,r]. t + b.s  with  t = WdT @ s,
s = WdT^T @ u + N*b,  u = sum_n X[:,n].
"""

import sys

sys.path.insert(0, "/opt/trn_rl_repo")

import numpy as np
import ml_dtypes

from concourse import bacc, tile, mybir
from concourse.bass_utils import run_bass_kernel_spmd

NCORES = 8
B, C, D, HWP = 16, 2048, 1024, 196
N = 3136          # B * 14 * 14 nodes
R = N // NCORES   # 392 local nodes = 2 images
NCLS = 80
KC = C // 128     # 16
KD = D // 128     # 8
MN = (N + 127) // 128  # 25 n-tiles (last has 64 rows)

F32 = mybir.dt.float32
BF16 = mybir.dt.bfloat16
RG = [list(range(NCORES))]

_BUILT = None


def _build():
    nc = bacc.Bacc("TRN2", target_bir_lowering=False, debug=False,
                   num_devices=NCORES)

    dp = nc.declare_dram_parameter
    xbf_d = dp("xbf", [C, R], BF16, isOutput=False)
    x32_d = dp("x32", [C, R], F32, isOutput=False)
    wdT_d = dp("wdT", [C, D], BF16, isOutput=False)
    w1_d = dp("w1", [D, D], BF16, isOutput=False)
    w2_d = dp("w2", [D, D], BF16, isOutput=False)
    wuT_d = dp("wuT", [D, C], BF16, isOutput=False)
    wf_d = dp("wfT", [C, NCLS], F32, isOutput=False)
    bd_d = dp("bd", [D, 1], F32, isOutput=False)
    b1_d = dp("b1", [D, 1], F32, isOutput=False)
    b2_d = dp("b2", [D, 1], F32, isOutput=False)
    bnA_d = dp("bnA", [C, 1], F32, isOutput=False)
    bnB_d = dp("bnB", [C, 1], F32, isOutput=False)
    bfc_d = dp("bfc", [NCLS, 1], F32, isOutput=False)
    t_d = dp("t32", [C, 1], F32, isOutput=False)
    bs_d = dp("bs", [1, 1], F32, isOutput=False)
    out_d = dp("out", [NCLS, 2], F32, isOutput=True)

    with tile.TileContext(nc) as tc:
        with (
            tc.tile_pool(name="wpool", bufs=1) as wp,
            tc.tile_pool(name="main", bufs=1) as mp,
            tc.tile_pool(name="dram", bufs=1, space="DRAM") as dr,
        ):
            # ---- long-lived SBUF tensors
            w1_sb = wp.tile([128, KD, D], BF16)
            w2_sb = wp.tile([128, KD, D], BF16)
            wuT_sb = wp.tile([128, KD, C], BF16)
            wf_sb = wp.tile([128, KC, NCLS], F32)
            bd_sb = wp.tile([128, KD], F32)
            b1_sb = wp.tile([128, KD], F32)
            b2_sb = wp.tile([128, KD], F32)
            bnA_sb = wp.tile([128, KC], F32)
            bnB_sb = wp.tile([128, KC], F32)
            bfc_sb = wp.tile([NCLS, 1], F32)
            t_sb = wp.tile([128, KC], F32)
            bs_sb = wp.tile([1, 1], F32)
            ones_sb = wp.tile([1, 128], F32)

            feat_bf = mp.tile([128, KD, R], BF16)
            adjT_sc = mp.tile([128, MN, R], BF16)
            gc1T = mp.tile([128, KD, R], BF16)
            gc2T = mp.tile([128, KD, R], BF16)
            rinv_bc = mp.tile([128, R], F32)
            rs_row = mp.tile([1, R], F32)
            rinv_row = mp.tile([1, R], F32)
            pooled = mp.tile([128, KC, 2], F32)
            s_sb = mp.tile([128, 4, D], BF16)  # support1 / support2 (reused)
            out_sb = mp.tile([NCLS, 2], F32)

            # ---- DRAM bounce buffers for collectives
            feat_bnc = dr.tile([D, R], BF16)
            feat_g = dr.tile([NCORES * D, R], BF16, addr_space="Shared")
            s1_bnc = dr.tile([R, D], BF16)
            s1_g = dr.tile([N, D], BF16, addr_space="Shared")
            s2_bnc = dr.tile([R, D], BF16)
            s2_g = dr.tile([N, D], BF16, addr_space="Shared")

            # round-robin DMA issue across engine queues
            _eng = [nc.sync, nc.scalar]
            _ei = [0]

            def dma(*a, **k):
                e = _eng[_ei[0] % len(_eng)]
                _ei[0] += 1
                return e.dma_start(*a, **k)

            # ---- phase 0: first-wave input DMAs (chunked so P1 starts early)
            with tc.tile_pool(name="downp", bufs=1) as dnp:
                xbf_sb = dnp.tile([128, KC, R], BF16)
                x32_sb = dnp.tile([128, KC, R], F32)
                wdT_sb = dnp.tile([128, KC, D], BF16)

                dma(bd_sb[:], bd_d.ap().rearrange("(k p) one -> p (k one)", p=128))
                xbf_r = xbf_d.ap().rearrange("(k p) r -> p k r", p=128)
                wdT_r = wdT_d.ap().rearrange("(k p) d -> p k d", p=128)
                for k in range(KC):
                    dma(wdT_sb[:, k, :], wdT_r[:, k, :])
                    dma(xbf_sb[:, k, :], xbf_r[:, k, :])
                w1_r = w1_d.ap().rearrange("(k p) d -> p k d", p=128)
                for k in range(KD):
                    dma(w1_sb[:, k, :], w1_r[:, k, :])
                dma(b1_sb[:], b1_d.ap().rearrange("(k p) one -> p (k one)", p=128))
                nc.vector.memset(ones_sb[:], 1.0)

                # ---- phase 1: conv1x1-down  feat_T[d, r] (bf16)
                with tc.tile_pool(name="ps1", bufs=1, space="PSUM") as ps1:
                    for m in range(KD):
                        pd = ps1.tile([128, R], F32, tag="down", bufs=3)
                        with tc.high_priority():
                            for k in range(KC):
                                nc.tensor.matmul(pd[:], wdT_sb[:, k, 128 * m:128 * (m + 1)],
                                                 xbf_sb[:, k, :],
                                                 start=(k == 0), stop=(k == KC - 1))
                            nc.vector.tensor_scalar_add(feat_bf[:, m, :], pd[:],
                                                        bd_sb[:, m:m + 1])
                            nc.gpsimd.dma_start(feat_bnc[128 * m:128 * (m + 1), :],
                                                feat_bf[:, m, :])

                    # ---- AllGather 1: feat
                    with tc.high_priority():
                        nc.gpsimd.collective_compute(
                            "AllGather", mybir.AluOpType.bypass, replica_groups=RG,
                            ins=[feat_bnc[:].opt()], outs=[feat_g[:].opt()])

                    # ---- phase 3: support1 = feat @ w1 (local rows), overlaps AG1
                    for t in range(4):
                        r0 = 128 * t
                        wt = min(128, R - r0)
                        for h in range(2):
                            p = ps1.tile([128, 512], F32, tag="s1", bufs=4)
                            for k in range(KD):
                                nc.tensor.matmul(
                                    p[:wt], feat_bf[:, k, r0:r0 + wt],
                                    w1_sb[:, k, 512 * h:512 * (h + 1)],
                                    start=(k == 0), stop=(k == KD - 1))
                            with tc.high_priority(offset=5000):
                                nc.vector.tensor_copy(
                                    s_sb[:wt, t, 512 * h:512 * (h + 1)], p[:wt])
                        with tc.high_priority(offset=5000):
                            nc.gpsimd.dma_start(s1_bnc[r0:r0 + wt, :],
                                                s_sb[:wt, t, :])

                    with tc.high_priority(offset=5000):
                        nc.gpsimd.collective_compute(
                            "AllGather", mybir.AluOpType.bypass, replica_groups=RG,
                            ins=[s1_bnc[:].opt()], outs=[s1_g[:].opt()])

                    # exact fp32 row sums of the (unnormalized) adjacency
                    x32_r = x32_d.ap().rearrange("(k p) r -> p k r", p=128)
                    for k in range(KC):
                        dma(x32_sb[:, k, :], x32_r[:, k, :])
                    dma(t_sb[:], t_d.ap().rearrange("(k p) one -> p (k one)", p=128))
                    dma(bs_sb[:], bs_d.ap())
                    prs = ps1.tile([1, R], F32, tag="rsbc")
                    for k in range(KC):
                        nc.tensor.matmul(prs[:], t_sb[:, k:k + 1], x32_sb[:, k, :],
                                         start=(k == 0), stop=(k == KC - 1))
                    nc.vector.tensor_scalar_add(rs_row[:], prs[:], bs_sb[:1, :])
                    nc.vector.reciprocal(rinv_row[:], rs_row[:])
                    pbc = ps1.tile([128, R], F32, tag="rsbc")
                    nc.tensor.matmul(pbc[:], ones_sb[:], rinv_row[:],
                                     start=True, stop=True)
                    nc.vector.tensor_copy(rinv_bc[:], pbc[:])

                # prefetch late-phase weights into the AG1 window
                w2_r = w2_d.ap().rearrange("(k p) d -> p k d", p=128)
                wuT_r = wuT_d.ap().rearrange("(k p) c -> p k c", p=128)
                wf_r = wf_d.ap().rearrange("(k p) o -> p k o", p=128)
                for k in range(KD):
                    dma(w2_sb[:, k, :], w2_r[:, k, :])
                dma(b2_sb[:], b2_d.ap().rearrange("(k p) one -> p (k one)", p=128))
                for k in range(KD):
                    dma(wuT_sb[:, k, :], wuT_r[:, k, :])
                for k in range(KC):
                    dma(wf_sb[:, k, :], wf_r[:, k, :])
                dma(bnA_sb[:], bnA_d.ap().rearrange("(k p) one -> p (k one)", p=128))
                dma(bnB_sb[:], bnB_d.ap().rearrange("(k p) one -> p (k one)", p=128))
                dma(bfc_sb[:], bfc_d.ap())

                # ---- phase 4: adjT (row-scaled), needs gathered feat
                with (
                    tc.tile_pool(name="ps4", bufs=1, space="PSUM") as ps4,
                    tc.tile_pool(name="adjlhs", bufs=4) as alp,
                ):
                    feat_g2 = feat_g[:].rearrange("(j k p) r -> j p k r", j=NCORES, p=128)
                    for m in range(MN):
                        n0 = 128 * m
                        w = min(128, N - n0)
                        pa = ps4.tile([128, R], F32, tag="adj", bufs=3)
                        # one 3-D DMA per block segment covers all 8 k-tiles
                        lt = alp.tile([128, KD, 128], BF16, tag="lt")
                        j0 = n0 // R
                        j1 = (n0 + w - 1) // R
                        for j in range(j0, j1 + 1):
                            a = max(n0, R * j)
                            b = min(n0 + w, R * (j + 1))
                            dma(lt[:, :, a - n0:b - n0],
                                feat_g2[j, :, :, a - R * j:b - R * j])
                        for k in range(KD):
                            nc.tensor.matmul(pa[:w], lt[:, k, :w], feat_bf[:, k, :],
                                             start=(k == 0), stop=(k == KD - 1))
                        nc.vector.tensor_tensor(adjT_sc[:w, m, :], pa[:w],
                                                rinv_bc[:w, :],
                                                op=mybir.AluOpType.mult)

            # ---- phase 5: gc1_T = relu((adj_sc @ support1)^T + b1)
            with (
                tc.tile_pool(name="ps5", bufs=1, space="PSUM") as ps5,
                tc.tile_pool(name="gclhs1", bufs=4) as glp1,
            ):
                pg = [ps5.tile([128, R], F32, tag=f"gc{m}", name=f"pgc{m}")
                      for m in range(KD)]
                for k in range(MN):
                    wk = min(128, N - 128 * k)
                    kt = glp1.tile([128, D], BF16, tag="kt")
                    dma(kt[:wk], s1_g[128 * k:128 * k + wk, :])
                    for m in range(KD):
                        nc.tensor.matmul(pg[m][:], kt[:wk, 128 * m:128 * (m + 1)],
                                         adjT_sc[:wk, k, :],
                                         start=(k == 0), stop=(k == MN - 1))
                for m in range(KD):
                    nc.scalar.activation(gc1T[:, m, :], pg[m][:],
                                         mybir.ActivationFunctionType.Relu,
                                         bias=b1_sb[:, m:m + 1], scale=1.0)

            # ---- phase 6: support2 = gc1 @ w2 (local rows)
            with tc.tile_pool(name="ps6", bufs=1, space="PSUM") as ps6:
                for t in range(4):
                    r0 = 128 * t
                    wt = min(128, R - r0)
                    for h in range(2):
                        p = ps6.tile([128, 512], F32, tag="s2", bufs=4)
                        for k in range(KD):
                            nc.tensor.matmul(
                                p[:wt], gc1T[:, k, r0:r0 + wt],
                                w2_sb[:, k, 512 * h:512 * (h + 1)],
                                start=(k == 0), stop=(k == KD - 1))
                        nc.vector.tensor_copy(
                            s_sb[:wt, t, 512 * h:512 * (h + 1)], p[:wt])
                    nc.gpsimd.dma_start(s2_bnc[r0:r0 + wt, :], s_sb[:wt, t, :])

            nc.gpsimd.collective_compute(
                "AllGather", mybir.AluOpType.bypass, replica_groups=RG,
                ins=[s2_bnc[:].opt()], outs=[s2_g[:].opt()])

            # ---- phase 7: gc2_T = relu((adj_sc @ support2)^T + b2)
            with (
                tc.tile_pool(name="ps7", bufs=1, space="PSUM") as ps7,
                tc.tile_pool(name="gclhs2", bufs=4) as glp2,
            ):
                pg = [ps7.tile([128, R], F32, tag=f"gd{m}", name=f"pgd{m}")
                      for m in range(KD)]
                for k in range(MN):
                    wk = min(128, N - 128 * k)
                    kt = glp2.tile([128, D], BF16, tag="kt")
                    dma(kt[:wk], s2_g[128 * k:128 * k + wk, :])
                    for m in range(KD):
                        nc.tensor.matmul(pg[m][:], kt[:wk, 128 * m:128 * (m + 1)],
                                         adjT_sc[:wk, k, :],
                                         start=(k == 0), stop=(k == MN - 1))
                for m in range(KD):
                    nc.scalar.activation(gc2T[:, m, :], pg[m][:],
                                         mybir.ActivationFunctionType.Relu,
                                         bias=b2_sb[:, m:m + 1], scale=1.0)

            # ---- phase 8: conv1x1-up + BN + residual + maxpool
            with (
                tc.tile_pool(name="ps8", bufs=1, space="PSUM") as ps8,
                tc.tile_pool(name="upp", bufs=1) as up,
            ):
                for m in range(KC):
                    pu = ps8.tile([128, R], F32, tag="up", bufs=3)
                    for k in range(KD):
                        nc.tensor.matmul(pu[:], wuT_sb[:, k, 128 * m:128 * (m + 1)],
                                         gc2T[:, k, :],
                                         start=(k == 0), stop=(k == KD - 1))
                    xb = up.tile([128, R], F32, tag="xbn", bufs=3)
                    nc.scalar.activation(xb[:], pu[:],
                                         mybir.ActivationFunctionType.Identity,
                                         bias=bnB_sb[:, m:m + 1],
                                         scale=bnA_sb[:, m:m + 1])
                    x32u = up.tile([128, R], F32, tag="x32u", bufs=3)
                    dma(x32u[:], x32_d[128 * m:128 * (m + 1), :])
                    xr = up.tile([128, R], F32, tag="xres", bufs=3)
                    nc.vector.tensor_tensor(xr[:], xb[:], x32u[:],
                                            op=mybir.AluOpType.add)
                    nc.vector.tensor_reduce(
                        pooled[:, m, :], xr[:].rearrange("p (i q) -> p i q", i=2),
                        axis=mybir.AxisListType.X, op=mybir.AluOpType.max)

                # ---- fc
                pfc = ps8.tile([NCLS, 2], F32, tag="fc")
                for k in range(KC):
                    nc.tensor.matmul(pfc[:], wf_sb[:, k, :], pooled[:, k, :],
                                     start=(k == 0), stop=(k == KC - 1))
                nc.scalar.activation(out_sb[:], pfc[:],
                                     mybir.ActivationFunctionType.Identity,
                                     bias=bfc_sb[:], scale=1.0)
                dma(out_d[:], out_sb[:])

    nc.compile()
    return nc


def _prep(inputs):
    bf = ml_dtypes.bfloat16
    f = np.ascontiguousarray(inputs["feature"], dtype=np.float32)
    X = np.ascontiguousarray(f.transpose(1, 0, 2, 3).reshape(C, N))

    wdT = np.ascontiguousarray(inputs["w_down"].T, dtype=np.float32)
    b_down = inputs["b_down"].astype(np.float64)

    # exact row-sum folding: row_sum[r] = X[:,r].t + b.s
    u = X.sum(1, dtype=np.float64)
    s = wdT.astype(np.float64).T @ u + N * b_down
    t = wdT.astype(np.float64) @ s
    bs = float(b_down @ s) + 1e-10

    A = (inputs["bn_gamma"] / np.sqrt(inputs["bn_var"] + 1e-5)).astype(np.float32)
    Bb = (inputs["bn_beta"] + (inputs["b_up"] - inputs["bn_mean"]) * A).astype(np.float32)

    com = {
        "wdT": wdT.astype(bf),
        "w1": np.ascontiguousarray(inputs["w1"], dtype=np.float32).astype(bf),
        "w2": np.ascontiguousarray(inputs["w2"], dtype=np.float32).astype(bf),
        "wuT": np.ascontiguousarray(inputs["w_up"].T, dtype=np.float32).astype(bf),
        "wfT": np.ascontiguousarray(inputs["w_fc"].T, dtype=np.float32),
        "bd": inputs["b_down"].astype(np.float32).reshape(D, 1),
        "b1": inputs["b1"].astype(np.float32).reshape(D, 1),
        "b2": inputs["b2"].astype(np.float32).reshape(D, 1),
        "bnA": A.reshape(C, 1),
        "bnB": Bb.reshape(C, 1),
        "bfc": inputs["b_fc"].astype(np.float32).reshape(NCLS, 1),
        "t32": t.astype(np.float32).reshape(C, 1),
        "bs": np.full((1, 1), bs, dtype=np.float32),
    }
    in_maps = []
    for c in range(NCORES):
        xl = np.ascontiguousarray(X[:, R * c:R * (c + 1)])
        m = dict(com)
        m["x32"] = xl
        m["xbf"] = xl.astype(bf)
        in_maps.append(m)
    return in_maps


def kernel(**inputs):
    global _BUILT
    if _BUILT is None:
        _BUILT = _build()
    in_maps = _prep(inputs)
    res = run_bass_kernel_spmd(_BUILT, in_maps, core_ids=list(range(NCORES)))
    out = np.empty((B, NCLS), dtype=np.float32)
    for c in range(NCORES):
        o = res.results[c]["out"]  # (NCLS, 2)
        out[2 * c] = o[:, 0]
        out[2 * c + 1] = o[:, 1]
    return out
